# revision 10
# baseline (speedup 1.0000x reference)
"""Trainium2 Bass kernel for nn_Attention1D (B=4, L=4096, C=64).

reference:
    Q = x@Wq + bq ; K = x@Wk + bk ; V = x@Wv + bv          (per batch b)
    s = Q @ K.T / sqrt(C)                                   [L_q, L_k]
    attn = softmax(s, axis=q)      # normalize over QUERY axis
    out = attn @ V + x

Sharding: 8 cores = 4 batches x 2 key-shards. softmax normalizes over q
(not sharded) so each core's softmax is local: Z[k] = sum_q exp(s[q,k]),
out += exp(s) @ (V/Z); the two k-shards' partial outputs ADD on the host,
which also transposes the channel-major core output and adds residual x.
Core (b, 1) gets x^T rolled by -2048 so its k-shard is always chunks 0-3;
the host unrolls its output.

Phase-split design (PSUM pools are sequential scopes so each phase gets
all 8 banks):
  Head: one combined-weights DMA + x^T in 8 chunks; early dummy ACTIVATE
    pre-loads the exp table; K/Q/V projections (contract 65 = bias row);
    Q evacs on ScalarE, K/V evacs on VectorE (parallel chains).
  Phase 1 conveyor: per (k-tile, q-half): 4 unpacked score MMs (contract
    64, N=512, f32r full rate) fill a [128,2048] PSUM slot (2-slot ring);
    slots drain in parallel on two engines:
      ScalarE: exp ACTIVATE -> ET bf16 + accum Z partial   (~2.25us)
      VectorE: Schraudolph exp: i16(s*K1+K2) IS bf16 exp2 bits (one
        tensor_scalar), then bf16 reduce for Z            (~3.7us)
    A 14-MM dummy burst at the end keeps the PE busy through the phase
    boundary so the HAM clock-gate enters phase 2 warm (2.4 GHz).
  Phase 2: AV with V-as-weights: outT[f,q] in 8 one-bank PSUM tiles
    [64,512]; j-outer/k-inner so each tile finishes early and its
    evac + DMA overlap the remaining MMs. Dense N=512 MMs run at the
    warm 216ns back-to-back rate.
"""

import numpy as np
import ml_dtypes  # noqa: F401  (np bf16 support registered on import)

B, L, C = 4, 4096, 64
NCORES = 8
KSH = L // 2          # k columns per core: 2048
NKT = KSH // 128      # 16 k-tiles per core
NQ5 = L // 512        # 8 q-chunks of 512
NCH = NKT * 2         # 32 score chunks of [128, 2048]

# Schraudolph exp in bf16: exp(x) ~= bitcast_bf16(i16(x*K1 + K2)).
# K1 = 2^7/ln2; K2 calibrated numerically (max rel err ~3.4%, robust to
# round/floor int conversion).
SCH_K1 = 128.0 / np.log(2.0)
SCH_K2 = 16250.75

# Columns of each [128,2048] score chunk drained by ScalarE exp; the rest
# (SCH_W) go to VectorE Schraudolph in parallel. 1536/512 balances the
# engines (ACT ~1.8us vs DVE ~1.7us incl the batched Z reduce).
ACT_W = 1536
SCH_W = 2048 - ACT_W

_cache = {}


def _build():
    import concourse.bacc as bacc
    import concourse.mybir as mybir
    import concourse.tile as tile

    bf16 = mybir.dt.bfloat16
    i16 = mybir.dt.int16
    f32 = mybir.dt.float32
    f32r = mybir.dt.float32r
    AF = mybir.ActivationFunctionType
    AX = mybir.AxisListType
    ALU = mybir.AluOpType

    nc = bacc.Bacc("TRN2", target_bir_lowering=False, debug=False)

    xt_d = nc.dram_tensor("xt", [C + 1, L], f32r, kind="ExternalInput")
    w_d = nc.dram_tensor("w", [C + 1, 3 * C], f32r, kind="ExternalInput")
    o_d = nc.dram_tensor("o", [C, L], f32, kind="ExternalOutput")

    with tile.TileContext(nc) as tc:
        with (
            tc.tile_pool(name="consts", bufs=1) as consts,
            tc.tile_pool(name="sb", bufs=1) as sb,
            tc.tile_pool(name="obp", bufs=2) as obp,
        ):
            # early exp-table preload: tiny ACTIVATE on a zeroed scratch
            scr = consts.tile([128, 8], f32)
            nc.vector.memset(scr, 0.0)
            nc.scalar.activation(out=scr, in_=scr, func=AF.Exp)

            wu = consts.tile([128, 512], bf16)   # warm-burst operand
            nc.vector.memset(wu, 0.0)

            w_s = consts.tile([C + 1, 3 * C], f32r)
            nc.sync.dma_start(out=w_s, in_=w_d.ap())
            wq_s = w_s[:, 0:C]
            wk_s = w_s[:, C:2 * C]
            wv_s = w_s[:, 2 * C:3 * C]

            xt_c = []
            for c in range(NQ5):
                t = sb.tile([C + 1, 512], f32r, tag=f"xt{c}")
                nc.sync.dma_start(out=t, in_=xt_d.ap()[:, c * 512:(c + 1) * 512])
                xt_c.append(t)

            qt_c = [sb.tile([64, 512], f32r, tag=f"qt{c}", name=f"qt{c}")
                    for c in range(NQ5)]
            kt_c = [sb.tile([64, 512], f32r, tag=f"kt{c}", name=f"kt{c}")
                    for c in range(4)]
            v_ts = [sb.tile([128, C], bf16, tag=f"v{k}", name=f"v{k}")
                    for k in range(NKT)]
            et_ts = [sb.tile([128, L], bf16, tag=f"et{k}", name=f"et{k}")
                     for k in range(NKT)]
            zpa = sb.tile([128, NCH], f32)    # ACT accum partials per chunk
            zpd = sb.tile([128, NCH], f32)    # DVE reduce partials per chunk
            za = sb.tile([128, NKT], f32)
            zd = sb.tile([128, NKT], f32)
            z_all = sb.tile([128, NKT], f32)
            rz_all = sb.tile([128, NKT], f32)
            gv_ts = [sb.tile([128, C], bf16, tag=f"gv{k}", name=f"gv{k}")
                     for k in range(NKT)]

            with tc.tile_pool(name="qkvp", bufs=4, space="PSUM") as qkvp:
                # K and Q (chunks 0-3) interleaved as x^T chunks land
                for c in range(4):
                    pk = qkvp.tile([128, 512], f32, tag="p", name=f"pk{c}")
                    nc.tensor.matmul(pk[0:64, :], lhsT=wk_s, rhs=xt_c[c],
                                     start=True, stop=True)
                    nc.vector.tensor_copy(out=kt_c[c], in_=pk[0:64, :])
                    pq = qkvp.tile([128, 512], f32, tag="p", name=f"pq{c}")
                    nc.tensor.matmul(pq[0:64, :], lhsT=wq_s, rhs=xt_c[c],
                                     start=True, stop=True)
                    nc.scalar.copy(out=qt_c[c], in_=pq[0:64, :])
                # V from the k-shard chunks 0-3 (ready early)
                for kt in range(NKT):
                    pv = qkvp.tile([128, C], f32, tag="v", name=f"pv{kt}")
                    nc.tensor.matmul(
                        pv,
                        lhsT=xt_c[kt // 4][:, (kt % 4) * 128:(kt % 4 + 1) * 128],
                        rhs=wv_s, start=True, stop=True,
                    )
                    nc.vector.tensor_copy(out=v_ts[kt], in_=pv)
                # Q chunks 4-7 (gated by the DMA tail)
                for c in range(4, NQ5):
                    pq = qkvp.tile([128, 512], f32, tag="p", name=f"pq{c}")
                    nc.tensor.matmul(pq[0:64, :], lhsT=wq_s, rhs=xt_c[c],
                                     start=True, stop=True)
                    nc.scalar.copy(out=qt_c[c], in_=pq[0:64, :])

            # ---------- phase 1: scores + exp conveyor ----------
            def z_batch(k0, k1):
                # Z = ACT partials + DVE partials for k-tiles [k0, k1)
                zva = zpa[:, 2 * k0:2 * k1].rearrange("p (k h) -> p k h", h=2)
                nc.vector.reduce_sum(out=za[:, k0:k1], in_=zva, axis=AX.X)
                zvd = zpd[:, 2 * k0:2 * k1].rearrange("p (k h) -> p k h", h=2)
                nc.vector.reduce_sum(out=zd[:, k0:k1], in_=zvd, axis=AX.X)
                nc.vector.tensor_tensor(
                    out=z_all[:, k0:k1], in0=za[:, k0:k1], in1=zd[:, k0:k1],
                    op=ALU.add,
                )
                nc.vector.reciprocal(out=rz_all[:, k0:k1], in_=z_all[:, k0:k1])
                for k2 in range(k0, k1):
                    nc.vector.tensor_scalar_mul(
                        gv_ts[k2], v_ts[k2], rz_all[:, k2:k2 + 1]
                    )

            with tc.tile_pool(name="scp", bufs=2, space="PSUM") as scp:
                for kt in range(NKT):
                    lk = kt_c[kt // 4][:, (kt % 4) * 128:(kt % 4 + 1) * 128]
                    for qh in range(2):
                        S = scp.tile([128, 2048], f32, tag="s")
                        for c5 in range(4):
                            nc.tensor.matmul(
                                S[:, c5 * 512:(c5 + 1) * 512],
                                lhsT=lk, rhs=qt_c[qh * 4 + c5],
                                start=True, stop=True,
                            )
                        ci = kt * 2 + qh
                        eslice = et_ts[kt][:, qh * 2048:(qh + 1) * 2048]
                        nc.scalar.activation(
                            out=eslice[:, 0:ACT_W], in_=S[:, 0:ACT_W],
                            func=AF.Exp, accum_out=zpa[:, ci:ci + 1],
                        )
                        nc.vector.tensor_scalar(
                            out=eslice[:, ACT_W:2048].bitcast(i16),
                            in0=S[:, ACT_W:2048],
                            scalar1=float(SCH_K1), scalar2=float(SCH_K2),
                            op0=ALU.mult, op1=ALU.add,
                        )
                    # batched Z-reduce of this k-tile's two Schraudolph tails
                    etv = et_ts[kt].rearrange("p (h q) -> p h q", h=2)
                    nc.vector.reduce_sum(
                        out=zpd[:, 2 * kt:2 * kt + 2],
                        in_=etv[:, :, ACT_W:2048], axis=AX.X,
                    )
                    if kt == 13:
                        # early Z/rz/gv for k-tiles 0..13: off the phase-2
                        # critical path (their partials are complete)
                        z_batch(0, 14)
                # keep the PE busy through the phase boundary (HAM warm)
                Sw = scp.tile([128, 2048], f32, tag="s")
                for i in range(14):
                    nc.tensor.matmul(
                        Sw[:, (i % 4) * 512:(i % 4 + 1) * 512],
                        lhsT=wu[:, 0:128], rhs=wu,
                        start=True, stop=True,
                    )

            z_batch(14, NKT)

            # ---------- phase 2: dense AV (V as weights, outT[f, q]) ----------
            o_ap = o_d.ap()
            with tc.tile_pool(name="accp", bufs=1, space="PSUM") as accp:
                acc = [accp.tile([128, 512], f32, tag=f"a{j}", name=f"a{j}")
                       for j in range(NQ5)]
                for j in range(NQ5):
                    for kt in range(NKT):
                        nc.tensor.matmul(
                            acc[j][0:64, :],
                            lhsT=gv_ts[kt],
                            rhs=et_ts[kt][:, j * 512:(j + 1) * 512],
                            start=(kt == 0), stop=(kt == NKT - 1),
                            skip_group_check=True,
                        )
                    ob = obp.tile([64, 512], f32, tag="ob")
                    nc.vector.tensor_copy(out=ob, in_=acc[j][0:64, :])
                    nc.sync.dma_start(
                        out=o_ap[:, j * 512:(j + 1) * 512], in_=ob,
                    )

    nc.compile()
    return nc


def _get_nc():
    if "nc" not in _cache:
        _cache["nc"] = _build()
    return _cache["nc"]


def _in_maps(x, Wq, bq, Wk, bk, Wv, bv):
    s = 1.0 / np.sqrt(np.float32(C))
    wq1 = (np.concatenate([Wq, bq[None, :]], 0) * s).astype(np.float32)
    wk1 = np.concatenate([Wk, bk[None, :]], 0).astype(np.float32)
    wv1 = np.concatenate([Wv, bv[None, :]], 0).astype(np.float32)
    w = np.ascontiguousarray(np.concatenate([wq1, wk1, wv1], 1))
    maps = []
    for core in range(NCORES):
        b, half = core // 2, core % 2
        x1t = np.ascontiguousarray(np.concatenate(
            [x[b], np.ones((L, 1), np.float32)], 1
        ).T.astype(np.float32))              # [65, L]
        # Roll so this core's k-shard sits in columns [0, KSH): the kernel
        # always takes k from chunks 0..3 and q from all 8 chunks.
        if half == 1:
            x1t = np.ascontiguousarray(np.roll(x1t, -KSH, axis=1))
        maps.append({"xt": x1t, "w": w})
    return maps


def _unshard(outs, x):
    full = np.empty((B, L, C), np.float32)
    for b in range(B):
        o0 = outs[2 * b].astype(np.float32)       # [C, L]
        o1 = outs[2 * b + 1].astype(np.float32)   # [C, L] rolled by -KSH
        o1 = np.roll(o1, KSH, axis=1)
        full[b] = (o0 + o1).T + x[b]
    return full


def _run(x, Wq, bq, Wk, bk, Wv, bv, trace=False):
    from concourse.bass_utils import run_bass_kernel_spmd

    nc = _get_nc()
    maps = _in_maps(x, Wq, bq, Wk, bk, Wv, bv)
    res = run_bass_kernel_spmd(
        nc, maps, core_ids=list(range(NCORES)), trace=trace
    )
    outs = [r["o"] for r in res.results]
    return _unshard(outs, x), res


def kernel(x, Wq, bq, Wk, bk, Wv, bv):
    x = np.asarray(x, np.float32)
    full, _ = _run(
        x,
        np.asarray(Wq, np.float32), np.asarray(bq, np.float32),
        np.asarray(Wk, np.float32), np.asarray(bk, np.float32),
        np.asarray(Wv, np.float32), np.asarray(bv, np.float32),
    )
    return full


# revision 14
# speedup vs baseline: 1.2290x; 1.2290x over previous
"""Trainium2 Bass kernel for nn_Attention1D (B=4, L=4096, C=64).

reference:
    Q = x@Wq + bq ; K = x@Wk + bk ; V = x@Wv + bv          (per batch b)
    s = Q @ K.T / sqrt(C)                                   [L_q, L_k]
    attn = softmax(s, axis=q)      # normalize over QUERY axis
    out = attn @ V + x

Sharding: 8 cores = 4 batches x 2 key-shards. softmax normalizes over q
(not sharded) so each core's softmax is local: Z[k] = sum_q exp(s[q,k]),
out += exp(s) @ (V/Z); the two k-shards' partial outputs ADD on the host,
which also transposes the channel-major core output and adds residual x.
Core (b, 1) gets x^T rolled by -2048 so its k-shard is always chunks 0-3;
the host unrolls its output.

Phase-split design (PSUM pools are sequential scopes so each phase gets
all 8 banks):
  Head: one combined-weights DMA + x^T in 8 chunks; early dummy ACTIVATE
    pre-loads the exp table; K/Q/V projections (contract 65 = bias row);
    Q evacs on ScalarE, K/V evacs on VectorE (parallel chains).
  Phase 1 conveyor: per (k-tile, q-half): 4 unpacked score MMs (contract
    64, N=512, f32r full rate) fill a [128,2048] PSUM slot (2-slot ring);
    slots drain in parallel on two engines:
      ScalarE: exp ACTIVATE -> ET bf16 + accum Z partial   (~2.25us)
      VectorE: Schraudolph exp: i16(s*K1+K2) IS bf16 exp2 bits (one
        tensor_scalar), then bf16 reduce for Z            (~3.7us)
    A 14-MM dummy burst at the end keeps the PE busy through the phase
    boundary so the HAM clock-gate enters phase 2 warm (2.4 GHz).
  Phase 2: AV with V-as-weights: outT[f,q] in 8 one-bank PSUM tiles
    [64,512]; j-outer/k-inner so each tile finishes early and its
    evac + DMA overlap the remaining MMs. Dense N=512 MMs run at the
    warm 216ns back-to-back rate.
"""

import numpy as np
import ml_dtypes  # noqa: F401  (np bf16 support registered on import)

B, L, C = 4, 4096, 64
NCORES = 8
KSH = L // 2          # k columns per core: 2048
NKT = KSH // 128      # 16 k-tiles per core
NQ5 = L // 512        # 8 q-chunks of 512
NCH = NKT * 2         # 32 score chunks of [128, 2048]

# Schraudolph exp in bf16: exp(x) ~= bitcast_bf16(i16(x*K1 + K2)).
# K1 = 2^7/ln2; K2 calibrated numerically (max rel err ~3.4%, robust to
# round/floor int conversion).
SCH_K1 = 128.0 / np.log(2.0)
SCH_K2 = 16250.75

# Score chunks drained whole by VectorE (Schraudolph tensor_scalar 2.3us
# + async bf16 Z-reduce); the rest go to ScalarE exp (2.25us). Each chunk
# owns its ET tile so the two engines never share a dependency range
# (a bitcast view defeats subtile tracking and would serialize them).
DVE_CHUNKS = frozenset(ci for ci in range(NCH) if ci % 4 == 1 and ci > 1)

_cache = {}


def _build():
    import concourse.bacc as bacc
    import concourse.mybir as mybir
    import concourse.tile as tile

    bf16 = mybir.dt.bfloat16
    i16 = mybir.dt.int16
    f32 = mybir.dt.float32
    f32r = mybir.dt.float32r
    AF = mybir.ActivationFunctionType
    AX = mybir.AxisListType
    ALU = mybir.AluOpType

    nc = bacc.Bacc("TRN2", target_bir_lowering=False, debug=False)

    xt_d = nc.dram_tensor("xt", [C + 1, L], f32r, kind="ExternalInput")
    w_d = nc.dram_tensor("w", [C + 1, 3 * C], f32r, kind="ExternalInput")
    o_d = nc.dram_tensor("o", [C, L], f32, kind="ExternalOutput")

    with tile.TileContext(nc) as tc:
        with (
            tc.tile_pool(name="consts", bufs=1) as consts,
            tc.tile_pool(name="sb", bufs=1) as sb,
            tc.tile_pool(name="obp", bufs=2) as obp,
        ):
            # early exp-table preload: tiny ACTIVATE on a zeroed scratch
            scr = consts.tile([128, 8], f32)
            nc.vector.memset(scr, 0.0)
            nc.scalar.activation(out=scr, in_=scr, func=AF.Exp)

            wu = consts.tile([128, 512], bf16)   # warm-burst operand
            nc.vector.memset(wu, 0.0)

            w_s = consts.tile([C + 1, 3 * C], f32r)
            nc.sync.dma_start(out=w_s, in_=w_d.ap())
            wq_s = w_s[:, 0:C]
            wk_s = w_s[:, C:2 * C]
            wv_s = w_s[:, 2 * C:3 * C]

            xt_c = []
            for c in range(NQ5):
                t = sb.tile([C + 1, 512], f32r, tag=f"xt{c}")
                nc.sync.dma_start(out=t, in_=xt_d.ap()[:, c * 512:(c + 1) * 512])
                xt_c.append(t)

            qt_c = [sb.tile([64, 512], f32r, tag=f"qt{c}", name=f"qt{c}")
                    for c in range(NQ5)]
            kt_c = [sb.tile([64, 512], f32r, tag=f"kt{c}", name=f"kt{c}")
                    for c in range(4)]
            v_ts = [sb.tile([128, C], bf16, tag=f"v{k}", name=f"v{k}")
                    for k in range(NKT)]
            et_ts = [sb.tile([128, 2048], bf16, tag=f"et{c}", name=f"et{c}")
                     for c in range(NCH)]     # one tile PER score chunk
            zp = sb.tile([128, NCH], f32)     # Z partials per chunk
            z_all = sb.tile([128, NKT], f32)
            rz_all = sb.tile([128, NKT], f32)
            gv_ts = [sb.tile([128, C], bf16, tag=f"gv{k}", name=f"gv{k}")
                     for k in range(NKT)]

            with tc.tile_pool(name="qkvp", bufs=4, space="PSUM") as qkvp:
                # K and Q (chunks 0-3) interleaved as x^T chunks land
                for c in range(4):
                    pk = qkvp.tile([128, 512], f32, tag="p", name=f"pk{c}")
                    nc.tensor.matmul(pk[0:64, :], lhsT=wk_s, rhs=xt_c[c],
                                     start=True, stop=True)
                    nc.vector.tensor_copy(out=kt_c[c], in_=pk[0:64, :])
                    pq = qkvp.tile([128, 512], f32, tag="p", name=f"pq{c}")
                    nc.tensor.matmul(pq[0:64, :], lhsT=wq_s, rhs=xt_c[c],
                                     start=True, stop=True)
                    nc.scalar.copy(out=qt_c[c], in_=pq[0:64, :])
                # V from the k-shard chunks 0-3 (ready early)
                for kt in range(NKT):
                    pv = qkvp.tile([128, C], f32, tag="v", name=f"pv{kt}")
                    nc.tensor.matmul(
                        pv,
                        lhsT=xt_c[kt // 4][:, (kt % 4) * 128:(kt % 4 + 1) * 128],
                        rhs=wv_s, start=True, stop=True,
                    )
                    nc.vector.tensor_copy(out=v_ts[kt], in_=pv)
                # Q chunks 4-7 (gated by the DMA tail)
                for c in range(4, NQ5):
                    pq = qkvp.tile([128, 512], f32, tag="p", name=f"pq{c}")
                    nc.tensor.matmul(pq[0:64, :], lhsT=wq_s, rhs=xt_c[c],
                                     start=True, stop=True)
                    nc.scalar.copy(out=qt_c[c], in_=pq[0:64, :])

            # ---------- phase 1: scores + exp conveyor ----------
            def z_batch(k0, k1):
                zv = zp[:, 2 * k0:2 * k1].rearrange("p (k h) -> p k h", h=2)
                nc.vector.reduce_sum(out=z_all[:, k0:k1], in_=zv, axis=AX.X)
                nc.vector.reciprocal(out=rz_all[:, k0:k1], in_=z_all[:, k0:k1])
                for k2 in range(k0, k1):
                    nc.vector.tensor_scalar_mul(
                        gv_ts[k2], v_ts[k2], rz_all[:, k2:k2 + 1]
                    )

            with tc.tile_pool(name="scp", bufs=2, space="PSUM") as scp:
                for kt in range(NKT):
                    lk = kt_c[kt // 4][:, (kt % 4) * 128:(kt % 4 + 1) * 128]
                    for qh in range(2):
                        S = scp.tile([128, 2048], f32, tag="s")
                        for c5 in range(4):
                            nc.tensor.matmul(
                                S[:, c5 * 512:(c5 + 1) * 512],
                                lhsT=lk, rhs=qt_c[qh * 4 + c5],
                                start=True, stop=True,
                            )
                        ci = kt * 2 + qh
                        if ci in DVE_CHUNKS:
                            nc.vector.tensor_scalar(
                                out=et_ts[ci].bitcast(i16), in0=S,
                                scalar1=float(SCH_K1), scalar2=float(SCH_K2),
                                op0=ALU.mult, op1=ALU.add,
                            )
                            nc.vector.reduce_sum(
                                out=zp[:, ci:ci + 1], in_=et_ts[ci], axis=AX.X,
                            )
                        else:
                            nc.scalar.activation(
                                out=et_ts[ci], in_=S, func=AF.Exp,
                                accum_out=zp[:, ci:ci + 1],
                            )
                    if kt == 13:
                        # early Z/rz/gv for k-tiles 0..13: off the phase-2
                        # critical path (their partials are complete)
                        z_batch(0, 14)
                # keep the PE busy through the phase boundary (HAM warm)
                Sw = scp.tile([128, 2048], f32, tag="s")
                for i in range(14):
                    nc.tensor.matmul(
                        Sw[:, (i % 4) * 512:(i % 4 + 1) * 512],
                        lhsT=wu[:, 0:128], rhs=wu,
                        start=True, stop=True,
                    )

            z_batch(14, NKT)

            # ---------- phase 2: dense AV (V as weights, outT[f, q]) ----------
            o_ap = o_d.ap()
            with tc.tile_pool(name="accp", bufs=1, space="PSUM") as accp:
                acc = [accp.tile([128, 512], f32, tag=f"a{j}", name=f"a{j}")
                       for j in range(NQ5)]
                for j in range(NQ5):
                    qh, jj = j // 4, j % 4
                    for kt in range(NKT):
                        nc.tensor.matmul(
                            acc[j][0:64, :],
                            lhsT=gv_ts[kt],
                            rhs=et_ts[2 * kt + qh][:, jj * 512:(jj + 1) * 512],
                            start=(kt == 0), stop=(kt == NKT - 1),
                            skip_group_check=True,
                        )
                    ob = obp.tile([64, 512], f32, tag="ob")
                    nc.vector.tensor_copy(out=ob, in_=acc[j][0:64, :])
                    nc.sync.dma_start(
                        out=o_ap[:, j * 512:(j + 1) * 512], in_=ob,
                    )

    nc.compile()
    return nc


def _get_nc():
    if "nc" not in _cache:
        _cache["nc"] = _build()
    return _cache["nc"]


def _in_maps(x, Wq, bq, Wk, bk, Wv, bv):
    s = 1.0 / np.sqrt(np.float32(C))
    wq1 = (np.concatenate([Wq, bq[None, :]], 0) * s).astype(np.float32)
    wk1 = np.concatenate([Wk, bk[None, :]], 0).astype(np.float32)
    wv1 = np.concatenate([Wv, bv[None, :]], 0).astype(np.float32)
    w = np.ascontiguousarray(np.concatenate([wq1, wk1, wv1], 1))
    maps = []
    for core in range(NCORES):
        b, half = core // 2, core % 2
        x1t = np.ascontiguousarray(np.concatenate(
            [x[b], np.ones((L, 1), np.float32)], 1
        ).T.astype(np.float32))              # [65, L]
        # Roll so this core's k-shard sits in columns [0, KSH): the kernel
        # always takes k from chunks 0..3 and q from all 8 chunks.
        if half == 1:
            x1t = np.ascontiguousarray(np.roll(x1t, -KSH, axis=1))
        maps.append({"xt": x1t, "w": w})
    return maps


def _unshard(outs, x):
    full = np.empty((B, L, C), np.float32)
    for b in range(B):
        o0 = outs[2 * b].astype(np.float32)       # [C, L]
        o1 = outs[2 * b + 1].astype(np.float32)   # [C, L] rolled by -KSH
        o1 = np.roll(o1, KSH, axis=1)
        full[b] = (o0 + o1).T + x[b]
    return full


def _run(x, Wq, bq, Wk, bk, Wv, bv, trace=False):
    from concourse.bass_utils import run_bass_kernel_spmd

    nc = _get_nc()
    maps = _in_maps(x, Wq, bq, Wk, bk, Wv, bv)
    res = run_bass_kernel_spmd(
        nc, maps, core_ids=list(range(NCORES)), trace=trace
    )
    outs = [r["o"] for r in res.results]
    return _unshard(outs, x), res


def kernel(x, Wq, bq, Wk, bk, Wv, bv):
    x = np.asarray(x, np.float32)
    full, _ = _run(
        x,
        np.asarray(Wq, np.float32), np.asarray(bq, np.float32),
        np.asarray(Wk, np.float32), np.asarray(bk, np.float32),
        np.asarray(Wv, np.float32), np.asarray(bv, np.float32),
    )
    return full


# revision 15
# speedup vs baseline: 1.2297x; 1.0006x over previous
"""Trainium2 Bass kernel for nn_Attention1D (B=4, L=4096, C=64).

reference:
    Q = x@Wq + bq ; K = x@Wk + bk ; V = x@Wv + bv          (per batch b)
    s = Q @ K.T / sqrt(C)                                   [L_q, L_k]
    attn = softmax(s, axis=q)      # normalize over QUERY axis
    out = attn @ V + x

Sharding: 8 cores = 4 batches x 2 key-shards. softmax normalizes over q
(not sharded) so each core's softmax is local: Z[k] = sum_q exp(s[q,k]),
out += exp(s) @ (V/Z); the two k-shards' partial outputs ADD on the host,
which also transposes the channel-major core output and adds residual x.
Core (b, 1) gets x^T rolled by -2048 so its k-shard is always chunks 0-3.

Phase-split design (sequential PSUM pools give each phase all 8 banks):
  Head: combined-weights DMA + x^T in 8 chunks; early dummy ACTIVATE
    preloads the exp table; projections evac through batched [128,1024]
    PSUM tiles -- Q on ScalarE, K/V on VectorE (parallel chains).
  Phase 1 conveyor: per (k-tile, q-half): 4 unpacked score MMs (contract
    65, N=512, f32r full rate ~427ns) fill a [128,2048] PSUM slot
    (2-slot ring). Slots drain 2048-wide on ScalarE (exp ACTIVATE +
    accum Z partial, ~2.25us) except DVE_RUNS chunk-runs, which drain on
    VectorE (Schraudolph: one tensor_scalar i16(s*K1+K2) whose bit
    pattern IS bf16 exp). Runs are consecutive so the engine-switch
    penalty (one exposed fill, ~2.1us) amortizes over 2-3 chunks. DVE
    Z-reduces are deferred into the following ACT-chunk stretch. Each
    chunk owns its ET tile (a bitcast view defeats subtile dep tracking,
    so sharing tiles between the engines would serialize them).
  Phase 2: dense AV, V-as-weights: outT[f,q] in 8 one-bank PSUM tiles
    [64,512], j-outer so each tile's evac+DMA overlap remaining MMs.
    A 14-MM dummy burst at the end of phase 1 keeps the PE busy through
    the pool boundary so the HAM clock-gate runs phase 2 warm (2.4GHz,
    216ns per N=512 MM).
"""

import numpy as np
import ml_dtypes  # noqa: F401  (np bf16 support registered on import)

B, L, C = 4, 4096, 64
NCORES = 8
KSH = L // 2          # k columns per core: 2048
NKT = KSH // 128      # 16 k-tiles per core
NQ5 = L // 512        # 8 q-chunks of 512
NCH = NKT * 2         # 32 score chunks of [128, 2048]

# Schraudolph exp in bf16: exp(x) ~= bitcast_bf16(i16(x*K1 + K2)).
# K1 = 2^7/ln2; K2 calibrated numerically (max rel err ~3.4%).
SCH_K1 = 128.0 / np.log(2.0)
SCH_K2 = 16250.75

# Chunk indices drained by VectorE, grouped in consecutive runs.
DVE_RUNS = ((6, 7, 8), (14, 15, 16), (22, 23, 24), (28, 29))
DVE_CHUNKS = frozenset(c for run in DVE_RUNS for c in run)

_cache = {}


def _build():
    import concourse.bacc as bacc
    import concourse.mybir as mybir
    import concourse.tile as tile

    bf16 = mybir.dt.bfloat16
    i16 = mybir.dt.int16
    f32 = mybir.dt.float32
    f32r = mybir.dt.float32r
    AF = mybir.ActivationFunctionType
    AX = mybir.AxisListType
    ALU = mybir.AluOpType

    nc = bacc.Bacc("TRN2", target_bir_lowering=False, debug=False)

    xt_d = nc.dram_tensor("xt", [C + 1, L], f32r, kind="ExternalInput")
    w_d = nc.dram_tensor("w", [C + 1, 3 * C], f32r, kind="ExternalInput")
    o_d = nc.dram_tensor("o", [C, L], f32, kind="ExternalOutput")

    with tile.TileContext(nc) as tc:
        with (
            tc.tile_pool(name="consts", bufs=1) as consts,
            tc.tile_pool(name="sb", bufs=1) as sb,
            tc.tile_pool(name="obp", bufs=2) as obp,
        ):
            # early exp-table preload: tiny ACTIVATE on a zeroed scratch
            scr = consts.tile([128, 8], f32)
            nc.vector.memset(scr, 0.0)
            nc.scalar.activation(out=scr, in_=scr, func=AF.Exp)

            wu = consts.tile([128, 512], bf16)   # warm-burst operand
            nc.vector.memset(wu, 0.0)

            w_s = consts.tile([C + 1, 3 * C], f32r)
            nc.sync.dma_start(out=w_s, in_=w_d.ap())
            wq_s = w_s[:, 0:C]
            wk_s = w_s[:, C:2 * C]
            wv_s = w_s[:, 2 * C:3 * C]

            xt_c = []
            for c in range(NQ5):
                t = sb.tile([C + 1, 512], f32r, tag=f"xt{c}")
                nc.sync.dma_start(out=t, in_=xt_d.ap()[:, c * 512:(c + 1) * 512])
                xt_c.append(t)

            qt_c = [sb.tile([64, 1024], f32r, tag=f"qt{c}", name=f"qt{c}")
                    for c in range(4)]
            kt_c = [sb.tile([64, 1024], f32r, tag=f"kt{c}", name=f"kt{c}")
                    for c in range(2)]
            v_all = sb.tile([128, NKT, C], bf16)
            et_ts = [sb.tile([128, 2048], bf16, tag=f"et{c}", name=f"et{c}")
                     for c in range(NCH)]     # one tile PER score chunk
            zp = sb.tile([128, NCH], f32)     # Z partials per chunk
            z_all = sb.tile([128, NKT], f32)
            rz_all = sb.tile([128, NKT], f32)
            gv_all = sb.tile([128, NKT, C], bf16)

            with tc.tile_pool(name="qkvp", bufs=1, space="PSUM") as qkvp:
                # K then Q interleaved as x^T chunks land; batched evacs:
                # Q copies on ScalarE, K/V casts on VectorE.
                for g in range(2):
                    pk = qkvp.tile([128, 1024], f32, tag="p", bufs=3,
                                   name=f"pk{g}")
                    for h in range(2):
                        nc.tensor.matmul(
                            pk[0:64, h * 512:(h + 1) * 512], lhsT=wk_s,
                            rhs=xt_c[2 * g + h], start=True, stop=True,
                        )
                    nc.vector.tensor_copy(out=kt_c[g], in_=pk[0:64, :])
                    pq = qkvp.tile([128, 1024], f32, tag="p", bufs=3,
                                   name=f"pq{g}")
                    for h in range(2):
                        nc.tensor.matmul(
                            pq[0:64, h * 512:(h + 1) * 512], lhsT=wq_s,
                            rhs=xt_c[2 * g + h], start=True, stop=True,
                        )
                    nc.scalar.copy(out=qt_c[g], in_=pq[0:64, :])
                # V from the k-shard chunks 0-3 (ready early)
                for g in range(4):
                    pv = qkvp.tile([128, 4, C], f32, tag="v", bufs=2,
                                   name=f"pv{g}")
                    for h in range(4):
                        kt = 4 * g + h
                        nc.tensor.matmul(
                            pv[:, h, :],
                            lhsT=xt_c[kt // 4][:, (kt % 4) * 128:(kt % 4 + 1) * 128],
                            rhs=wv_s, start=True, stop=True,
                        )
                    nc.vector.tensor_copy(out=v_all[:, 4 * g:4 * (g + 1), :],
                                          in_=pv)
                # Q chunks 4-7 (gated by the DMA tail)
                for g in range(2, 4):
                    pq = qkvp.tile([128, 1024], f32, tag="p", bufs=3,
                                   name=f"pq{g}")
                    for h in range(2):
                        nc.tensor.matmul(
                            pq[0:64, h * 512:(h + 1) * 512], lhsT=wq_s,
                            rhs=xt_c[2 * g + h], start=True, stop=True,
                        )
                    nc.scalar.copy(out=qt_c[g], in_=pq[0:64, :])

            # ---------- phase 1: scores + exp conveyor ----------
            def z_batch(k0, k1):
                zv = zp[:, 2 * k0:2 * k1].rearrange("p (k h) -> p k h", h=2)
                nc.vector.reduce_sum(out=z_all[:, k0:k1], in_=zv, axis=AX.X)
                nc.vector.reciprocal(out=rz_all[:, k0:k1], in_=z_all[:, k0:k1])
                for k2 in range(k0, k1):
                    nc.vector.tensor_scalar_mul(
                        gv_all[:, k2, :], v_all[:, k2, :], rz_all[:, k2:k2 + 1]
                    )

            pending_rd = []

            def flush_rd(n):
                for _ in range(n):
                    if not pending_rd:
                        return
                    c = pending_rd.pop(0)
                    nc.vector.reduce_sum(
                        out=zp[:, c:c + 1], in_=et_ts[c], axis=AX.X,
                    )

            with tc.tile_pool(name="scp", bufs=2, space="PSUM") as scp:
                for kt in range(NKT):
                    lk = kt_c[kt // 8][:, (kt % 8) * 128:(kt % 8 + 1) * 128]
                    for qh in range(2):
                        S = scp.tile([128, 2048], f32, tag="s")
                        for c5 in range(4):
                            cc = qh * 4 + c5
                            nc.tensor.matmul(
                                S[:, c5 * 512:(c5 + 1) * 512],
                                lhsT=lk,
                                rhs=qt_c[cc // 2][:, (cc % 2) * 512:(cc % 2 + 1) * 512],
                                start=True, stop=True,
                            )
                        ci = kt * 2 + qh
                        if ci in DVE_CHUNKS:
                            nc.vector.tensor_scalar(
                                out=et_ts[ci].bitcast(i16), in0=S,
                                scalar1=float(SCH_K1), scalar2=float(SCH_K2),
                                op0=ALU.mult, op1=ALU.add,
                            )
                            pending_rd.append(ci)
                        else:
                            nc.scalar.activation(
                                out=et_ts[ci], in_=S, func=AF.Exp,
                                accum_out=zp[:, ci:ci + 1],
                            )
                            flush_rd(1)
                    if kt == 13:
                        # early Z/rz/gv for k-tiles 0..12 (all partials in)
                        z_batch(0, 13)
                # keep the PE busy through the phase boundary (HAM warm)
                Sw = scp.tile([128, 2048], f32, tag="s")
                for i in range(14):
                    nc.tensor.matmul(
                        Sw[:, (i % 4) * 512:(i % 4 + 1) * 512],
                        lhsT=wu[:, 0:128], rhs=wu,
                        start=True, stop=True,
                    )

            flush_rd(len(pending_rd))
            z_batch(13, NKT)

            # ---------- phase 2: dense AV (V as weights, outT[f, q]) ----------
            o_ap = o_d.ap()
            with tc.tile_pool(name="accp", bufs=1, space="PSUM") as accp:
                acc = [accp.tile([128, 512], f32, tag=f"a{j}", name=f"a{j}")
                       for j in range(NQ5)]
                for j in range(NQ5):
                    qh, jj = j // 4, j % 4
                    for kt in range(NKT):
                        nc.tensor.matmul(
                            acc[j][0:64, :],
                            lhsT=gv_all[:, kt, :],
                            rhs=et_ts[2 * kt + qh][:, jj * 512:(jj + 1) * 512],
                            start=(kt == 0), stop=(kt == NKT - 1),
                            skip_group_check=True,
                        )
                    ob = obp.tile([64, 512], f32, tag="ob")
                    nc.vector.tensor_copy(out=ob, in_=acc[j][0:64, :])
                    nc.sync.dma_start(
                        out=o_ap[:, j * 512:(j + 1) * 512], in_=ob,
                    )

    nc.compile()
    return nc


def _get_nc():
    if "nc" not in _cache:
        _cache["nc"] = _build()
    return _cache["nc"]


def _in_maps(x, Wq, bq, Wk, bk, Wv, bv):
    s = 1.0 / np.sqrt(np.float32(C))
    wq1 = (np.concatenate([Wq, bq[None, :]], 0) * s).astype(np.float32)
    wk1 = np.concatenate([Wk, bk[None, :]], 0).astype(np.float32)
    wv1 = np.concatenate([Wv, bv[None, :]], 0).astype(np.float32)
    w = np.ascontiguousarray(np.concatenate([wq1, wk1, wv1], 1))
    maps = []
    for core in range(NCORES):
        b, half = core // 2, core % 2
        x1t = np.ascontiguousarray(np.concatenate(
            [x[b], np.ones((L, 1), np.float32)], 1
        ).T.astype(np.float32))              # [65, L]
        if half == 1:
            x1t = np.ascontiguousarray(np.roll(x1t, -KSH, axis=1))
        maps.append({"xt": x1t, "w": w})
    return maps


def _unshard(outs, x):
    full = np.empty((B, L, C), np.float32)
    for b in range(B):
        o0 = outs[2 * b].astype(np.float32)       # [C, L]
        o1 = outs[2 * b + 1].astype(np.float32)   # [C, L] rolled by -KSH
        o1 = np.roll(o1, KSH, axis=1)
        full[b] = (o0 + o1).T + x[b]
    return full


def _run(x, Wq, bq, Wk, bk, Wv, bv, trace=False):
    from concourse.bass_utils import run_bass_kernel_spmd

    nc = _get_nc()
    maps = _in_maps(x, Wq, bq, Wk, bk, Wv, bv)
    res = run_bass_kernel_spmd(
        nc, maps, core_ids=list(range(NCORES)), trace=trace
    )
    outs = [r["o"] for r in res.results]
    return _unshard(outs, x), res


def kernel(x, Wq, bq, Wk, bk, Wv, bv):
    x = np.asarray(x, np.float32)
    full, _ = _run(
        x,
        np.asarray(Wq, np.float32), np.asarray(bq, np.float32),
        np.asarray(Wk, np.float32), np.asarray(bk, np.float32),
        np.asarray(Wv, np.float32), np.asarray(bv, np.float32),
    )
    return full
